# revision 1
# baseline (speedup 1.0000x reference)
"""FFT-based linear convolution of two 2^23-point real signals on 8 trn2 NeuronCores.

Math: conv(a, x) = Im(ifft(fft(a + i*x)^2)) / 2, with the 2^24-point FFT done as a
3-factor (256^3) matmul FFT. Stage A (over n1) is computed r-sharded across cores,
one AllToAll reshards to k1-sharded for the middle row-FFTs (stages B, C), the
pointwise square happens in the digit-reversed domain, then the inverse stages
(C', B') run locally, a second AllToAll reshards back, and inverse stage A'
produces only the imaginary part of the first half of the time-domain signal.

v2: DMA batching (CH=1024, interleaved T1 table, plane-merged loads/stores),
middle phase processes k1 in pairs with fused [128,512] elementwise ops and
512-wide moving matmuls in stages C/B', elementwise work spread over DVE/Pool/ACT.
"""
import os
import numpy as np

os.environ.setdefault("JAX_PLATFORMS", "")
import jax

jax.config.update("jax_compilation_cache_dir", "/tmp/jax_neff_cache")
jax.config.update("jax_persistent_cache_min_entry_size_bytes", -1)
jax.config.update("jax_persistent_cache_min_compile_time_secs", 0)

import concourse.bass as bass
import concourse.tile as tile
from concourse import bacc, mybir
from concourse.bass_utils import run_bass_kernel_spmd

N = 8388608          # input length
M = 2 * N            # FFT size = 2^24
B = 256              # radix
R = B * B            # 65536
W = 8                # cores
RL = R // W          # 8192 columns of r per core
CH = 1024            # free-dim chunk in stages A / A'
NCHUNK = RL // CH    # 8
KG = 16              # middle-phase k1l pair groups (2 k1l each)
F32 = mybir.dt.float32

USE_F32R = True
MMD = mybir.dt.float32r if USE_F32R else F32
BF16 = mybir.dt.bfloat16


def build_nc():
    nc = bacc.Bacc("TRN2", target_bir_lowering=False, debug=False, num_devices=W)

    a_in = nc.dram_tensor("a_c", [128, RL], BF16, kind="ExternalInput")
    x_in = nc.dram_tensor("x_c", [128, RL], BF16, kind="ExternalInput")
    # interleaved twiddle: [256, NCHUNK, 2 (re/im), CH]
    t1c_in = nc.dram_tensor("t1c", [B, NCHUNK * 2 * CH], BF16, kind="ExternalInput")
    dr_in = nc.dram_tensor("dr", [B, B], MMD, kind="ExternalInput")
    di_in = nc.dram_tensor("di", [B, B], MMD, kind="ExternalInput")
    ndi_in = nc.dram_tensor("ndi", [B, B], MMD, kind="ExternalInput")
    t2r_in = nc.dram_tensor("t2r", [B, B], F32, kind="ExternalInput")
    t2i_in = nc.dram_tensor("t2i", [B, B], F32, kind="ExternalInput")
    aw1_in = nc.dram_tensor("aw1", [B, 128], MMD, kind="ExternalInput")
    aw2_in = nc.dram_tensor("aw2", [B, 128], MMD, kind="ExternalInput")
    y_out = nc.dram_tensor("y_c", [128, RL], F32, kind="ExternalOutput")

    rg = [list(range(W))]

    with tile.TileContext(nc) as tc:
        with tc.tile_pool(name="dram", bufs=1, space="DRAM") as dram, \
             tc.tile_pool(name="consts", bufs=1) as consts:
            cc1_in = dram.tile([W, 32, 2, RL], BF16)
            cc1_out = dram.tile([W, 32, 2, RL], BF16)
            cc2_in = dram.tile([W, 32, 2, 32, B], BF16)
            cc2_out = dram.tile([W, 32, 2, 32, B], BF16)

            # ---- constant tables in SBUF ----
            dr_row, di_row, ndi_row = [], [], []
            drb_row, dib_row, ndib_row = [], [], []  # bf16 copies (stage-B moving)
            for p in range(2):
                for lst, src in ((dr_row, dr_in), (di_row, di_in), (ndi_row, ndi_in)):
                    t = consts.tile([128, B], MMD, name=f"c_{src.name}_{p}", tag=f"c_{src.name}_{p}")
                    nc.sync.dma_start(t[:], src[128 * p:128 * (p + 1), :])
                    lst.append(t)
                for nm, lst, srct in (("drb", drb_row, dr_row), ("dib", dib_row, di_row),
                                      ("ndib", ndib_row, ndi_row)):
                    t = consts.tile([128, B], BF16, name=f"c_{nm}_{p}", tag=f"c_{nm}_{p}")
                    nc.scalar.copy(t[:], srct[p][:])
                    lst.append(t)
            aw1_blk, aw2_blk = [], []
            for p in range(2):
                for lst, src in ((aw1_blk, aw1_in), (aw2_blk, aw2_in)):
                    t = consts.tile([128, 128], MMD, name=f"c_{src.name}_{p}", tag=f"c_{src.name}_{p}")
                    nc.sync.dma_start(t[:], src[128 * p:128 * (p + 1), :])
                    lst.append(t)
            # concatenated moving tables for fused [re|im] matmuls:
            # stage B (bf16): catB1=[dr|di], catB2=[ndi|dr]
            # stage C' (f32r): catC1=[dr|ndi], catC2=[di|dr]
            # twiddle cats (f32): t2ri=[t2r|t2i], t2ir=[t2i|t2r]
            catB1, catB2, catC1, catC2, t2ri, t2ir = [], [], [], [], [], []
            for p in range(2):
                for nm, lst, h0, h1 in (("catB1", catB1, drb_row, dib_row),
                                        ("catB2", catB2, ndib_row, drb_row)):
                    t = consts.tile([128, 2 * B], BF16, name=f"c_{nm}_{p}", tag=f"c_{nm}_{p}")
                    nc.scalar.copy(t[:, 0:B], h0[p][:])
                    nc.scalar.copy(t[:, B:2 * B], h1[p][:])
                    lst.append(t)
                for nm, lst, s0, s1 in (("catC1", catC1, dr_in, ndi_in),
                                        ("catC2", catC2, di_in, dr_in)):
                    t = consts.tile([128, 2 * B], MMD, name=f"c_{nm}_{p}", tag=f"c_{nm}_{p}")
                    nc.sync.dma_start(t[:, 0:B], s0[128 * p:128 * (p + 1), :])
                    nc.sync.dma_start(t[:, B:2 * B], s1[128 * p:128 * (p + 1), :])
                    lst.append(t)
                for nm, lst, s0, s1 in (("t2ri", t2ri, t2r_in, t2i_in),
                                        ("t2ir", t2ir, t2i_in, t2r_in)):
                    t = consts.tile([128, 2 * B], F32, name=f"c_{nm}_{p}", tag=f"c_{nm}_{p}")
                    nc.sync.dma_start(t[:, 0:B], s0[128 * p:128 * (p + 1), :])
                    nc.sync.dma_start(t[:, B:2 * B], s1[128 * p:128 * (p + 1), :])
                    lst.append(t)

            # ================= Phase A: stage A + T1 twiddle =================
            with tc.tile_pool(name="a_io", bufs=1) as a_io, \
                 tc.tile_pool(name="a_t1", bufs=4) as a_t1, \
                 tc.tile_pool(name="a_tmp", bufs=12) as a_tmp, \
                 tc.tile_pool(name="a_out", bufs=4) as a_outp, \
                 tc.tile_pool(name="a_ps", bufs=4, space="PSUM") as a_ps:
                a_full = a_io.tile([128, RL], BF16)
                nc.sync.dma_start(a_full[:], a_in[:, :])
                x_full = a_io.tile([128, RL], BF16)
                nc.sync.dma_start(x_full[:], x_in[:, :])

                for c in range(NCHUNK):
                    a_sl = a_full[:, c * CH:(c + 1) * CH]
                    x_sl = x_full[:, c * CH:(c + 1) * CH]
                    for h in range(2):
                        hs = slice(128 * h, 128 * (h + 1))
                        ps_r = a_ps.tile([128, CH], F32, tag="ps")
                        ps_i = a_ps.tile([128, CH], F32, tag="ps")
                        for q in range(2):
                            qs = slice(q * 512, (q + 1) * 512)
                            nc.tensor.matmul(ps_r[:, qs], drb_row[0][:, hs], a_sl[:, qs],
                                             start=True, stop=False)
                            nc.tensor.matmul(ps_i[:, qs], drb_row[0][:, hs], x_sl[:, qs],
                                             start=True, stop=False)
                            nc.tensor.matmul(ps_r[:, qs], ndib_row[0][:, hs], x_sl[:, qs],
                                             start=False, stop=True)
                            nc.tensor.matmul(ps_i[:, qs], dib_row[0][:, hs], a_sl[:, qs],
                                             start=False, stop=True)

                        t1_t = a_t1.tile([128, 2 * CH], BF16, tag="t1")
                        nc.sync.dma_start(t1_t[:], t1c_in[hs, c * 2 * CH:(c + 1) * 2 * CH])
                        t1r_t = t1_t[:, 0:CH]
                        t1i_t = t1_t[:, CH:2 * CH]

                        # Y' = (ps_r + i ps_i) * (t1r + i t1i), packed [Re | Im]
                        out_t = a_outp.tile([128, 2 * CH], BF16, tag="aout")
                        m1 = a_tmp.tile([128, CH], F32, tag="tmp")
                        m2 = a_tmp.tile([128, CH], F32, tag="tmp")
                        m3 = a_tmp.tile([128, CH], F32, tag="tmp")
                        m4 = a_tmp.tile([128, CH], F32, tag="tmp")
                        nc.vector.tensor_mul(m1[:], ps_r[:], t1r_t)
                        nc.vector.tensor_mul(m2[:], ps_i[:], t1i_t)
                        nc.vector.tensor_mul(m3[:], ps_r[:], t1i_t)
                        nc.vector.tensor_mul(m4[:], ps_i[:], t1r_t)
                        nc.gpsimd.tensor_sub(out_t[:, 0:CH], m1[:], m2[:])
                        nc.gpsimd.tensor_add(out_t[:, CH:2 * CH], m3[:], m4[:])

                        # store: dims (j=4, k1l=32, plane=2, rl=CH)
                        nc.sync.dma_start(
                            cc1_in[4 * h:4 * (h + 1), :, :, c * CH:(c + 1) * CH],
                            out_t[:])

            nc.gpsimd.collective_compute(
                "AllToAll", mybir.AluOpType.bypass, replica_groups=rg,
                ins=[cc1_in.opt()], outs=[cc1_out.opt()])

            # ============ Middle: per-k1-pair row FFT + square ============
            with tc.tile_pool(name="m_in", bufs=48) as m_in, \
                 tc.tile_pool(name="m_sb", bufs=18) as m_sb, \
                 tc.tile_pool(name="m_out", bufs=8) as m_out, \
                 tc.tile_pool(name="m_ps", bufs=8, space="PSUM") as m_ps:
                for kg in range(KG):
                    # load Y[k1] as (n2, n3) per (kk, n2h, plane) — v1 layout
                    y_t = []  # [kk][n2h][plane]
                    for kk in range(2):
                        rows = []
                        for n2h in range(2):
                            row = []
                            for pl in range(2):
                                t = m_in.tile([128, B], BF16, tag="yin")
                                nc.sync.dma_start(
                                    t[:], cc1_out[4 * n2h:4 * (n2h + 1), 2 * kg + kk, pl, :])
                                row.append(t)
                            rows.append(row)
                        y_t.append(rows)

                    # stage B (data as weights, fused [zr|zi] moving) + T2 twiddle
                    zt_sb = []  # [n3h] -> (ztr, zti) fused (kk, k2) [128, 2B]
                    for n3h in range(2):
                        ztr = m_sb.tile([128, 2 * B], MMD, tag="zt")
                        zti = m_sb.tile([128, 2 * B], MMD, tag="zt")
                        for kk in range(2):
                            ks = slice(kk * B, (kk + 1) * B)
                            z_f = m_ps.tile([128, 2 * B], F32, tag="mps")
                            for n2h in range(2):
                                st = n2h == 0
                                sp = n2h == 1
                                yre = y_t[kk][n2h][0][:, 128 * n3h:128 * n3h + 128]
                                yim = y_t[kk][n2h][1][:, 128 * n3h:128 * n3h + 128]
                                nc.tensor.matmul(z_f[:], yre, catB1[n2h][:],
                                                 start=st, stop=False, skip_group_check=True)
                                nc.tensor.matmul(z_f[:], yim, catB2[n2h][:],
                                                 start=False, stop=sp, skip_group_check=True)
                            p1 = m_sb.tile([128, 2 * B], F32, tag="mtmp")
                            p2 = m_sb.tile([128, 2 * B], F32, tag="mtmp")
                            nc.vector.tensor_mul(p1[:], z_f[:], t2ri[n3h][:])
                            nc.vector.tensor_mul(p2[:], z_f[:], t2ir[n3h][:])
                            nc.gpsimd.tensor_sub(ztr[:, ks], p1[:, 0:B], p1[:, B:2 * B])
                            nc.gpsimd.tensor_add(zti[:, ks], p2[:, 0:B], p2[:, B:2 * B])
                        zt_sb.append((ztr, zti))

                    # stage C (DFT stationary, 512-wide moving): U^T (k3, (kk, k2))
                    ut_ps = []
                    for k3h in range(2):
                        ks = slice(128 * k3h, 128 * (k3h + 1))
                        ur = m_ps.tile([128, 2 * B], F32, tag="mps")
                        ui = m_ps.tile([128, 2 * B], F32, tag="mps")
                        for n3h in range(2):
                            st = n3h == 0
                            sp = n3h == 1
                            nc.tensor.matmul(ur[:], dr_row[n3h][:, ks], zt_sb[n3h][0][:],
                                             start=st, stop=False, skip_group_check=True)
                            nc.tensor.matmul(ui[:], di_row[n3h][:, ks], zt_sb[n3h][0][:],
                                             start=st, stop=False, skip_group_check=True)
                            nc.tensor.matmul(ur[:], ndi_row[n3h][:, ks], zt_sb[n3h][1][:],
                                             start=False, stop=sp, skip_group_check=True)
                            nc.tensor.matmul(ui[:], dr_row[n3h][:, ks], zt_sb[n3h][1][:],
                                             start=False, stop=sp, skip_group_check=True)
                        ut_ps.append((ur, ui))

                    # square: S = U^2 (k3, (kk, k2)) -> SBUF, fused pair
                    s_sb = []
                    for k3h in range(2):
                        ur, ui = ut_ps[k3h]
                        sr = m_sb.tile([128, 2 * B], MMD, tag="ssb")
                        si = m_sb.tile([128, 2 * B], MMD, tag="ssb")
                        uc = m_sb.tile([128, 2 * B], F32, tag="mtmp")
                        q1 = m_sb.tile([128, 2 * B], F32, tag="mtmp")
                        q2 = m_sb.tile([128, 2 * B], F32, tag="mtmp")
                        nc.scalar.copy(uc[:], ur[:])
                        nc.vector.tensor_add(q1[:], uc[:], ui[:])
                        nc.vector.tensor_sub(q2[:], uc[:], ui[:])
                        nc.vector.scalar_tensor_tensor(
                            si[:], uc[:], 2.0, ui[:],
                            mybir.AluOpType.mult, mybir.AluOpType.mult)
                        nc.gpsimd.tensor_mul(sr[:], q1[:], q2[:])
                        s_sb.append((sr, si))

                    # stage C' (data as weights, fused [z2r|z2i] moving) + conj(T2)
                    y2_sb = []  # [k2h] -> (y2r, y2i) fused (kk, n3) [128, 2B]
                    for k2h in range(2):
                        y2r = m_sb.tile([128, 2 * B], MMD, tag="y2")
                        y2i = m_sb.tile([128, 2 * B], MMD, tag="y2")
                        for kk in range(2):
                            ks = slice(kk * B, (kk + 1) * B)
                            z2_f = m_ps.tile([128, 2 * B], F32, tag="mps")
                            for k3h in range(2):
                                st = k3h == 0
                                sp = k3h == 1
                                sre = s_sb[k3h][0][:, kk * B + 128 * k2h: kk * B + 128 * k2h + 128]
                                sim = s_sb[k3h][1][:, kk * B + 128 * k2h: kk * B + 128 * k2h + 128]
                                nc.tensor.matmul(z2_f[:], sre, catC1[k3h][:],
                                                 start=st, stop=False, skip_group_check=True)
                                nc.tensor.matmul(z2_f[:], sim, catC2[k3h][:],
                                                 start=False, stop=sp, skip_group_check=True)
                            p1 = m_sb.tile([128, 2 * B], F32, tag="mtmp")
                            p2 = m_sb.tile([128, 2 * B], F32, tag="mtmp")
                            nc.vector.tensor_mul(p1[:], z2_f[:], t2ri[k2h][:])
                            nc.vector.tensor_mul(p2[:], z2_f[:], t2ir[k2h][:])
                            nc.gpsimd.tensor_add(y2r[:, ks], p1[:, 0:B], p1[:, B:2 * B])
                            nc.gpsimd.tensor_sub(y2i[:, ks], p2[:, B:2 * B], p2[:, 0:B])
                        y2_sb.append((y2r, y2i))

                    # stage B' (DFT stationary, conj D, 512-wide moving): Y' (n2, (kk, n3))
                    for n2h in range(2):
                        ns = slice(128 * n2h, 128 * (n2h + 1))
                        yr = m_ps.tile([128, 2 * B], F32, tag="mps")
                        yi = m_ps.tile([128, 2 * B], F32, tag="mps")
                        for k2h in range(2):
                            st = k2h == 0
                            sp = k2h == 1
                            nc.tensor.matmul(yr[:], dr_row[k2h][:, ns], y2_sb[k2h][0][:],
                                             start=st, stop=False, skip_group_check=True)
                            nc.tensor.matmul(yi[:], dr_row[k2h][:, ns], y2_sb[k2h][1][:],
                                             start=st, stop=False, skip_group_check=True)
                            nc.tensor.matmul(yr[:], di_row[k2h][:, ns], y2_sb[k2h][1][:],
                                             start=False, stop=sp, skip_group_check=True)
                            nc.tensor.matmul(yi[:], ndi_row[k2h][:, ns], y2_sb[k2h][0][:],
                                             start=False, stop=sp, skip_group_check=True)
                        # copy fused (kk, n3) rows to SBUF, store per (plane, kk)
                        for pl, ps in ((0, yr), (1, yi)):
                            o = m_out.tile([128, 2 * B], BF16, tag="mout")
                            nc.scalar.copy(o[:], ps[:])
                            for kk in range(2):
                                nc.sync.dma_start(
                                    cc2_in[4 * n2h:4 * (n2h + 1), 2 * kg + kk, pl, :, :],
                                    o[:, kk * B:(kk + 1) * B])

            nc.gpsimd.collective_compute(
                "AllToAll", mybir.AluOpType.bypass, replica_groups=rg,
                ins=[cc2_in.opt()], outs=[cc2_out.opt()])

            # ============ Phase A': conj(T1), inverse stage A (Im only) ============
            NL = CH // B  # n2l values per chunk
            with tc.tile_pool(name="f_in", bufs=8) as f_in, \
                 tc.tile_pool(name="f_t1", bufs=2) as f_t1, \
                 tc.tile_pool(name="f_tmp", bufs=10) as f_tmp, \
                 tc.tile_pool(name="f_out", bufs=4) as f_outp, \
                 tc.tile_pool(name="f_ps", bufs=4, space="PSUM") as f_ps:
                for c in range(NCHUNK):
                    ps_o = f_ps.tile([128, CH], F32, tag="fps")
                    for h in range(2):
                        hs = slice(128 * h, 128 * (h + 1))
                        pp = f_in.tile([128, 2 * CH], BF16, tag="pin")
                        nc.sync.dma_start(
                            pp[:], cc2_out[4 * h:4 * (h + 1), :, :, NL * c:NL * (c + 1), :])
                        pr = pp[:, 0:CH]
                        pi = pp[:, CH:2 * CH]
                        t1_t = f_t1.tile([128, 2 * CH], BF16, tag="ft1")
                        nc.sync.dma_start(t1_t[:], t1c_in[hs, c * 2 * CH:(c + 1) * 2 * CH])
                        t1r_t = t1_t[:, 0:CH]
                        t1i_t = t1_t[:, CH:2 * CH]

                        # Yf = P * conj(T1)
                        yfr = f_tmp.tile([128, CH], MMD, tag="yf")
                        yfi = f_tmp.tile([128, CH], MMD, tag="yf")
                        p1 = f_tmp.tile([128, CH], F32, tag="ftmp")
                        p2 = f_tmp.tile([128, CH], F32, tag="ftmp")
                        p3 = f_tmp.tile([128, CH], F32, tag="ftmp")
                        p4 = f_tmp.tile([128, CH], F32, tag="ftmp")
                        nc.vector.tensor_mul(p1[:], pr, t1r_t)
                        nc.gpsimd.tensor_mul(p2[:], pi, t1i_t)
                        nc.vector.tensor_mul(p3[:], pi, t1r_t)
                        nc.gpsimd.tensor_mul(p4[:], pr, t1i_t)
                        nc.vector.tensor_add(yfr[:], p1[:], p2[:])
                        nc.vector.tensor_sub(yfi[:], p3[:], p4[:])

                        st = h == 0
                        sp = h == 1
                        for q in range(2):
                            qs = slice(q * 512, (q + 1) * 512)
                            nc.tensor.matmul(ps_o[:, qs], aw1_blk[h][:], yfi[:, qs],
                                             start=st, stop=False, skip_group_check=True)
                            nc.tensor.matmul(ps_o[:, qs], aw2_blk[h][:], yfr[:, qs],
                                             start=False, stop=sp, skip_group_check=True)

                    o = f_outp.tile([128, CH], F32, tag="fout")
                    nc.scalar.copy(o[:], ps_o[:])
                    nc.sync.dma_start(y_out[:, c * CH:(c + 1) * CH], o[:])

    nc.compile()
    return nc


_NC = None
_TABLES = None


def _tables():
    global _TABLES
    if _TABLES is None:
        k = np.arange(B)
        D = np.exp(-2j * np.pi * np.outer(k, k) / B)
        T2 = np.exp(-2j * np.pi * np.outer(k, k) / R)
        s = 1.0 / (2.0 * M)
        dr = np.ascontiguousarray(D.real.astype(np.float32))
        di = np.ascontiguousarray(D.imag.astype(np.float32))
        t1s = []
        for c in range(W):
            r = np.arange(c * RL, (c + 1) * RL)
            T1 = np.exp(-2j * np.pi * np.outer(k, r) / M)
            import ml_dtypes
            t1r = T1.real.astype(np.float32).reshape(B, NCHUNK, CH)
            t1i = T1.imag.astype(np.float32).reshape(B, NCHUNK, CH)
            t1c = np.empty((B, NCHUNK, 2, CH), np.float32)
            t1c[:, :, 0, :] = t1r
            t1c[:, :, 1, :] = t1i
            t1s.append(np.ascontiguousarray(
                t1c.reshape(B, NCHUNK * 2 * CH).astype(ml_dtypes.bfloat16)))
        _TABLES = dict(
            dr=dr, di=di, ndi=np.ascontiguousarray(-di),
            t2r=np.ascontiguousarray(T2.real.astype(np.float32)),
            t2i=np.ascontiguousarray(T2.imag.astype(np.float32)),
            aw1=np.ascontiguousarray((s * D.real[:, :128]).astype(np.float32)),
            aw2=np.ascontiguousarray((-s * D.imag[:, :128]).astype(np.float32)),
            t1s=t1s,
        )
    return _TABLES


def make_in_maps(a, x):
    tb = _tables()
    a3 = a.reshape(128, W, RL)
    x3 = x.reshape(128, W, RL)
    import ml_dtypes
    in_maps = []
    for c in range(W):
        in_maps.append(dict(
            a_c=np.ascontiguousarray(a3[:, c, :].astype(ml_dtypes.bfloat16)),
            x_c=np.ascontiguousarray(x3[:, c, :].astype(ml_dtypes.bfloat16)),
            t1c=tb["t1s"][c],
            dr=tb["dr"], di=tb["di"], ndi=tb["ndi"],
            t2r=tb["t2r"], t2i=tb["t2i"],
            aw1=tb["aw1"], aw2=tb["aw2"],
        ))
    return in_maps


def kernel(a, x, _want_trace=False, **_unused):
    global _NC
    a = np.asarray(a, dtype=np.float32)
    x = np.asarray(x, dtype=np.float32)
    if _NC is None:
        _NC = build_nc()
    in_maps = make_in_maps(a, x)
    res = run_bass_kernel_spmd(_NC, in_maps, core_ids=list(range(W)),
                               trace=_want_trace)
    full = np.empty((128, R), dtype=np.float32)
    for c in range(W):
        full[:, c * RL:(c + 1) * RL] = res.results[c]["y_c"]
    out = full.reshape(-1)
    if _want_trace:
        return out, res
    return out



# revision 31
# speedup vs baseline: 78.1682x; 78.1682x over previous
"""FFT-based linear convolution of two 2^23-point real signals on 8 trn2 NeuronCores.

Math: conv(a, x) = Im(ifft(fft(a + i*x)^2)) / 2, with the 2^24-point FFT done as a
3-factor (256^3) matmul FFT. Stage A (over n1) is computed r-sharded across cores,
one AllToAll reshards to k1-sharded for the middle row-FFTs (stages B, C), the
pointwise square happens in the digit-reversed domain, then the inverse stages
(C', B') run locally, a second AllToAll reshards back, and inverse stage A'
produces only the imaginary part of the first half of the time-domain signal.

v3: DMA-issue-count optimization. The v2 kernel was bound by the SP engine
serially issuing ~356 DMA instructions (~1.7us each in the cost model). v3:
  - all f32/f32r/bf16 constant tables packed host-side into 3 DRAM tensors
    (3 loads instead of 26), T1 twiddle packed as one [128, 32768] tensor
    (1 load instead of 32, resident in SBUF for both phase A and A').
  - middle phase goes through DRAM->DRAM corner-turn ("relayout") passes around
    the AllToAlls, so its SBUF loads/stores are large p-major contiguous tiles:
    (32 relayout + 16 load + 16 store + 32 relayout) vs 256 small DMAs.
  - phase A stores / A' loads merged to chunk-pair granularity (8 each).
  - second AllToAll split in 2 chunks overlapped with middle-phase compute;
    DMA issue spread across both HW DGE queues (SP + Activation).
"""
import os
import numpy as np

os.environ.setdefault("JAX_PLATFORMS", "")
import jax

jax.config.update("jax_compilation_cache_dir", "/tmp/jax_neff_cache")
jax.config.update("jax_persistent_cache_min_entry_size_bytes", -1)
jax.config.update("jax_persistent_cache_min_compile_time_secs", 0)

import concourse.bass as bass
import concourse.tile as tile
from concourse import bacc, mybir
from concourse.bass_utils import run_bass_kernel_spmd

N = 8388608          # input length
M = 2 * N            # FFT size = 2^24
B = 256              # radix
R = B * B            # 65536
W = 8                # cores
RL = R // W          # 8192 columns of r per core
CH = 1024            # free-dim chunk in stages A / A'
NCHUNK = RL // CH    # 8
NG = 8               # middle-phase k1l groups (4 k1l each)
F32 = mybir.dt.float32
MMD = mybir.dt.float32r
BF16 = mybir.dt.bfloat16


def build_nc():
    nc = bacc.Bacc("TRN2", target_bir_lowering=False, debug=False, num_devices=W)

    a_in = nc.dram_tensor("a_c", [128, RL], BF16, kind="ExternalInput")
    x_in = nc.dram_tensor("x_c", [128, RL], BF16, kind="ExternalInput")
    # T1 twiddle, packed [128, h, chunk, plane, CH]
    t1_in = nc.dram_tensor("t1c", [128, 2, NCHUNK, 2, CH], BF16, kind="ExternalInput")
    # packed const tables (see _tables for column layout)
    ckr_in = nc.dram_tensor("ckr", [128, 4096], MMD, kind="ExternalInput")
    ckf_in = nc.dram_tensor("ckf", [128, 2048], F32, kind="ExternalInput")
    ckb_in = nc.dram_tensor("ckb", [128, 3328], BF16, kind="ExternalInput")
    y_out = nc.dram_tensor("y_c", [128, RL], F32, kind="ExternalOutput")

    rg = [list(range(W))]

    with tile.TileContext(nc) as tc:
        with tc.tile_pool(name="dram", bufs=1, space="DRAM") as dram, \
             tc.tile_pool(name="consts", bufs=1) as consts:
            # per-chunk AllToAll tensors (collective APs must be contiguous)
            cc1_in = [dram.tile([W, 32, 2, 2 * CH], BF16, name=f"cc1i{c}") for c in range(4)]
            cc1_out = [dram.tile([W, 32, 2, 2 * CH], BF16, name=f"cc1o{c}") for c in range(4)]
            cc2_in = [dram.tile([W, 16, 2, RL], BF16, name=f"cc2i{q}") for q in range(2)]
            cc2_out = [dram.tile([W, 16, 2, RL], BF16, name=f"cc2o{q}") for q in range(2)]
            # middle-friendly (p-major, plane-major) mirrors of cc1_out / cc2_in
            ccM = dram.tile([2, 128, 2, 32, B], BF16)
            ccM2 = dram.tile([2, 128, 2, 32, B], BF16)

            # ---- packed constant tables in SBUF ----
            ckr_t = consts.tile([128, 4096], MMD)
            nc.sync.dma_start(ckr_t[:], ckr_in[:, :])
            ckf_t = consts.tile([128, 2048], F32)
            nc.sync.dma_start(ckf_t[:], ckf_in[:, :])
            ckb_t = consts.tile([128, 3328], BF16)
            nc.scalar.dma_start(ckb_t[:], ckb_in[:, :])
            t1_t = consts.tile([128, 2, NCHUNK, 2, CH], BF16)

            # const views
            def dr_v(p, s):      # D.real rows half p, col slice s
                return ckr_t[:, 256 * p + s.start:256 * p + s.stop]
            def di_v(p, s):
                return ckr_t[:, 512 + 256 * p + s.start:512 + 256 * p + s.stop]
            def ndi_v(p, s):
                return ckr_t[:, 1024 + 256 * p + s.start:1024 + 256 * p + s.stop]
            catC1 = [ckr_t[:, 1536 + 512 * p:1536 + 512 * (p + 1)] for p in range(2)]
            catC2 = [ckr_t[:, 2560 + 512 * p:2560 + 512 * (p + 1)] for p in range(2)]
            aw1_blk = [ckr_t[:, 3584 + 128 * h:3584 + 128 * (h + 1)] for h in range(2)]
            aw2_blk = [ckr_t[:, 3840 + 128 * h:3840 + 128 * (h + 1)] for h in range(2)]
            t2ri = [ckf_t[:, 512 * p:512 * (p + 1)] for p in range(2)]
            t2ir = [ckf_t[:, 1024 + 512 * p:1024 + 512 * (p + 1)] for p in range(2)]
            drb0 = [ckb_t[:, 128 * h:128 * (h + 1)] for h in range(2)]
            dib0 = [ckb_t[:, 256 + 128 * h:256 + 128 * (h + 1)] for h in range(2)]
            ndib0 = [ckb_t[:, 512 + 128 * h:512 + 128 * (h + 1)] for h in range(2)]
            catB1 = [ckb_t[:, 768 + 512 * p:768 + 512 * (p + 1)] for p in range(2)]
            catB2 = [ckb_t[:, 1792 + 512 * p:1792 + 512 * (p + 1)] for p in range(2)]
            aw1b = [ckb_t[:, 2816 + 128 * h:2816 + 128 * (h + 1)] for h in range(2)]
            aw2b = [ckb_t[:, 3072 + 128 * h:3072 + 128 * (h + 1)] for h in range(2)]

            # ================= Phase A: stage A + T1 twiddle =================
            with tc.tile_pool(name="a_io", bufs=1) as a_io, \
                 tc.tile_pool(name="a_tmp", bufs=8) as a_tmp, \
                 tc.tile_pool(name="a_out", bufs=2) as a_outp, \
                 tc.tile_pool(name="a_ps", bufs=4, space="PSUM") as a_ps:
                a_full = a_io.tile([128, RL], BF16)
                x_full = a_io.tile([128, RL], BF16)

                for c2 in range(NCHUNK // 2):
                    # chunked input/t1 loads so compute starts early
                    cs = slice(2 * CH * c2, 2 * CH * (c2 + 1))
                    nc.sync.dma_start(a_full[:, cs], a_in[:, cs])
                    nc.scalar.dma_start(x_full[:, cs], x_in[:, cs])
                    nc.sync.dma_start(t1_t[:, :, 2 * c2:2 * c2 + 2, :, :],
                                      t1_in[:, :, 2 * c2:2 * c2 + 2, :, :])
                    out2 = [a_outp.tile([128, 2, 2 * CH], BF16, name=f"aout{h}", tag=f"aout{h}")
                            for h in range(2)]
                    for cc in range(2):
                        c = 2 * c2 + cc
                        a_sl = a_full[:, c * CH:(c + 1) * CH]
                        x_sl = x_full[:, c * CH:(c + 1) * CH]
                        for h in range(2):
                            ps_r = a_ps.tile([128, CH], F32, tag="ps")
                            ps_i = a_ps.tile([128, CH], F32, tag="ps")
                            for q in range(2):
                                qs = slice(q * 512, (q + 1) * 512)
                                nc.tensor.matmul(ps_r[:, qs], drb0[h], a_sl[:, qs],
                                                 start=True, stop=False)
                                nc.tensor.matmul(ps_i[:, qs], drb0[h], x_sl[:, qs],
                                                 start=True, stop=False)
                                nc.tensor.matmul(ps_r[:, qs], ndib0[h], x_sl[:, qs],
                                                 start=False, stop=True)
                                nc.tensor.matmul(ps_i[:, qs], dib0[h], a_sl[:, qs],
                                                 start=False, stop=True)

                            t1r_t = t1_t[:, h, c, 0, :]
                            t1i_t = t1_t[:, h, c, 1, :]
                            # Y' = (ps_r + i ps_i) * (t1r + i t1i), in bf16 so the
                            # DVE runs in its 2-byte fast mode (PSUM cast via Act)
                            br = a_tmp.tile([128, CH], BF16, tag="cst")
                            bi = a_tmp.tile([128, CH], BF16, tag="cst")
                            nc.scalar.copy(br[:], ps_r[:])
                            nc.scalar.copy(bi[:], ps_i[:])
                            m1 = a_tmp.tile([128, CH], BF16, tag="tmp")
                            m2 = a_tmp.tile([128, CH], BF16, tag="tmp")
                            m3 = a_tmp.tile([128, CH], BF16, tag="tmp")
                            m4 = a_tmp.tile([128, CH], BF16, tag="tmp")
                            nc.vector.tensor_mul(m1[:], br[:], t1r_t)
                            nc.vector.tensor_mul(m2[:], bi[:], t1i_t)
                            nc.vector.tensor_mul(m3[:], br[:], t1i_t)
                            nc.vector.tensor_mul(m4[:], bi[:], t1r_t)
                            nc.vector.tensor_sub(
                                out2[h][:, 0, cc * CH:(cc + 1) * CH], m1[:], m2[:])
                            nc.vector.tensor_add(
                                out2[h][:, 1, cc * CH:(cc + 1) * CH], m3[:], m4[:])
                    for h in range(2):
                        for kh in range(2):
                            eng = nc.sync if (h + kh) % 2 == 0 else nc.scalar
                            eng.dma_start(
                                cc1_in[c2][4 * h:4 * (h + 1), 16 * kh:16 * (kh + 1), :, :],
                                out2[h][64 * kh:64 * (kh + 1), :, :])
                    nc.gpsimd.collective_compute(
                        "AllToAll", mybir.AluOpType.bypass, replica_groups=rg,
                        ins=[cc1_in[c2][:, :, :, :].rearrange("w k p j -> w (k p j)")],
                        outs=[cc1_out[c2][:, :, :, :].rearrange("w k p j -> w (k p j)")])

            # relayout 1: cc1_out [w, c2, k1l, pl, (mc n3)] -> ccM [n2h, p=(d,m), pl, k1l, n3]
            for c2 in range(4):
                for n2h in range(2):
                    for pl in range(2):
                        for d in range(4):
                            sv = cc1_out[c2][4 * n2h + d, :, pl, :] \
                                .rearrange("k (m n) -> m k n", m=8, n=B)
                            dv = ccM[n2h, 32 * d + 8 * c2:32 * d + 8 * (c2 + 1), pl, :, :]
                            eng = nc.sync if (pl + d + c2) % 2 == 0 else nc.scalar
                            eng.dma_start(dv, sv)

            # ============ Middle: per-k1-pair row FFT + square ============
            with tc.tile_pool(name="m_in", bufs=2) as m_in, \
                 tc.tile_pool(name="m_zt", bufs=8) as m_zt, \
                 tc.tile_pool(name="m_tmp", bufs=14) as m_tmp, \
                 tc.tile_pool(name="m_ssb", bufs=6) as m_ssb, \
                 tc.tile_pool(name="m_y2", bufs=6) as m_y2, \
                 tc.tile_pool(name="m_out", bufs=2) as m_out, \
                 tc.tile_pool(name="m_ps", bufs=8, space="PSUM") as m_ps:
                for g in range(NG):
                    yin_t = []
                    for n2h in range(2):
                        t = m_in.tile([128, 2, 4, B], BF16, name=f"yin{n2h}", tag=f"yin{n2h}")
                        eng = nc.sync if n2h == 0 else nc.scalar
                        eng.dma_start(t[:, :, :, :], ccM[n2h, :, :, 4 * g:4 * (g + 1), :])
                        yin_t.append(t)
                    yout_t = [m_out.tile([128, 2, 4, B], BF16, name=f"yout{n2h}", tag=f"yout{n2h}")
                              for n2h in range(2)]

                    for kgl in range(2):
                        # stage B (data as weights, fused [zr|zi] moving) + T2 twiddle
                        zt_sb = []  # [n3h] -> (ztr, zti) fused (kk, k2) [128, 2B]
                        for n3h in range(2):
                            ztr = m_zt.tile([128, 2 * B], MMD, tag="zt")
                            zti = m_zt.tile([128, 2 * B], MMD, tag="zt")
                            for kk in range(2):
                                ks = slice(kk * B, (kk + 1) * B)
                                z_f = m_ps.tile([128, 2 * B], F32, tag="mps")
                                for n2h in range(2):
                                    st = n2h == 0
                                    sp = n2h == 1
                                    yre = yin_t[n2h][:, 0, 2 * kgl + kk,
                                                     128 * n3h:128 * n3h + 128]
                                    yim = yin_t[n2h][:, 1, 2 * kgl + kk,
                                                     128 * n3h:128 * n3h + 128]
                                    nc.tensor.matmul(z_f[:], yre, catB1[n2h],
                                                     start=st, stop=False,
                                                     skip_group_check=True)
                                    nc.tensor.matmul(z_f[:], yim, catB2[n2h],
                                                     start=False, stop=sp,
                                                     skip_group_check=True)
                                p1 = m_tmp.tile([128, 2 * B], F32, tag="mtmp")
                                p2 = m_tmp.tile([128, 2 * B], F32, tag="mtmp")
                                nc.vector.tensor_mul(p1[:], z_f[:], t2ri[n3h])
                                nc.vector.tensor_mul(p2[:], z_f[:], t2ir[n3h])
                                nc.gpsimd.tensor_sub(ztr[:, ks], p1[:, 0:B], p1[:, B:2 * B])
                                nc.gpsimd.tensor_add(zti[:, ks], p2[:, 0:B], p2[:, B:2 * B])
                            zt_sb.append((ztr, zti))

                        # stage C (DFT stationary, 512-wide moving): U^T (k3, (kk, k2))
                        ut_ps = []
                        for k3h in range(2):
                            ks = slice(128 * k3h, 128 * (k3h + 1))
                            ur = m_ps.tile([128, 2 * B], F32, tag="mps")
                            ui = m_ps.tile([128, 2 * B], F32, tag="mps")
                            for n3h in range(2):
                                st = n3h == 0
                                sp = n3h == 1
                                nc.tensor.matmul(ur[:], dr_v(n3h, ks), zt_sb[n3h][0][:],
                                                 start=st, stop=False, skip_group_check=True)
                                nc.tensor.matmul(ui[:], di_v(n3h, ks), zt_sb[n3h][0][:],
                                                 start=st, stop=False, skip_group_check=True)
                                nc.tensor.matmul(ur[:], ndi_v(n3h, ks), zt_sb[n3h][1][:],
                                                 start=False, stop=sp, skip_group_check=True)
                                nc.tensor.matmul(ui[:], dr_v(n3h, ks), zt_sb[n3h][1][:],
                                                 start=False, stop=sp, skip_group_check=True)
                            ut_ps.append((ur, ui))

                        # square: S = U^2 (k3, (kk, k2)) -> SBUF, fused pair
                        s_sb = []
                        for k3h in range(2):
                            ur, ui = ut_ps[k3h]
                            sr = m_ssb.tile([128, 2 * B], MMD, tag="ssb")
                            si = m_ssb.tile([128, 2 * B], MMD, tag="ssb")
                            uc = m_tmp.tile([128, 2 * B], F32, tag="mtmp")
                            q1 = m_tmp.tile([128, 2 * B], F32, tag="mtmp")
                            q2 = m_tmp.tile([128, 2 * B], F32, tag="mtmp")
                            nc.scalar.copy(uc[:], ur[:])
                            nc.vector.tensor_add(q1[:], uc[:], ui[:])
                            nc.vector.tensor_sub(q2[:], uc[:], ui[:])
                            nc.vector.scalar_tensor_tensor(
                                si[:], uc[:], 2.0, ui[:],
                                mybir.AluOpType.mult, mybir.AluOpType.mult)
                            nc.gpsimd.tensor_mul(sr[:], q1[:], q2[:])
                            s_sb.append((sr, si))

                        # stage C' (data as weights, fused moving) + conj(T2)
                        y2_sb = []  # [k2h] -> (y2r, y2i) fused (kk, n3) [128, 2B]
                        for k2h in range(2):
                            y2r = m_y2.tile([128, 2 * B], MMD, tag="y2")
                            y2i = m_y2.tile([128, 2 * B], MMD, tag="y2")
                            for kk in range(2):
                                ks = slice(kk * B, (kk + 1) * B)
                                z2_f = m_ps.tile([128, 2 * B], F32, tag="mps")
                                for k3h in range(2):
                                    st = k3h == 0
                                    sp = k3h == 1
                                    sre = s_sb[k3h][0][:, kk * B + 128 * k2h:
                                                       kk * B + 128 * k2h + 128]
                                    sim = s_sb[k3h][1][:, kk * B + 128 * k2h:
                                                       kk * B + 128 * k2h + 128]
                                    nc.tensor.matmul(z2_f[:], sre, catC1[k3h],
                                                     start=st, stop=False,
                                                     skip_group_check=True)
                                    nc.tensor.matmul(z2_f[:], sim, catC2[k3h],
                                                     start=False, stop=sp,
                                                     skip_group_check=True)
                                p1 = m_tmp.tile([128, 2 * B], F32, tag="mtmp")
                                p2 = m_tmp.tile([128, 2 * B], F32, tag="mtmp")
                                nc.vector.tensor_mul(p1[:], z2_f[:], t2ri[k2h])
                                nc.vector.tensor_mul(p2[:], z2_f[:], t2ir[k2h])
                                nc.gpsimd.tensor_add(y2r[:, ks], p1[:, 0:B], p1[:, B:2 * B])
                                nc.gpsimd.tensor_sub(y2i[:, ks], p2[:, B:2 * B], p2[:, 0:B])
                            y2_sb.append((y2r, y2i))

                        # stage B' (DFT stationary, conj D): Y' (n2, (kk, n3))
                        for n2h in range(2):
                            ns = slice(128 * n2h, 128 * (n2h + 1))
                            yr = m_ps.tile([128, 2 * B], F32, tag="mps")
                            yi = m_ps.tile([128, 2 * B], F32, tag="mps")
                            for k2h in range(2):
                                st = k2h == 0
                                sp = k2h == 1
                                nc.tensor.matmul(yr[:], dr_v(k2h, ns), y2_sb[k2h][0][:],
                                                 start=st, stop=False, skip_group_check=True)
                                nc.tensor.matmul(yi[:], dr_v(k2h, ns), y2_sb[k2h][1][:],
                                                 start=st, stop=False, skip_group_check=True)
                                nc.tensor.matmul(yr[:], di_v(k2h, ns), y2_sb[k2h][1][:],
                                                 start=False, stop=sp, skip_group_check=True)
                                nc.tensor.matmul(yi[:], ndi_v(k2h, ns), y2_sb[k2h][0][:],
                                                 start=False, stop=sp, skip_group_check=True)
                            nc.scalar.copy(yout_t[n2h][:, 0, 2 * kgl:2 * kgl + 2, :], yr[:])
                            nc.scalar.copy(yout_t[n2h][:, 1, 2 * kgl:2 * kgl + 2, :], yi[:])

                    for n2h in range(2):
                        eng = nc.scalar if n2h == 0 else nc.sync
                        eng.dma_start(ccM2[n2h, :, :, 4 * g:4 * (g + 1), :],
                                      yout_t[n2h][:, :, :, :])

                    if g in (3, 7):
                        kh = g // 4
                        # relayout 2: ccM2 -> cc2_in[kh] [w, k1l_h, pl, rl]
                        for n2h in range(2):
                            for pl in range(2):
                                for d in range(4):
                                    sv = ccM2[n2h, 32 * d:32 * (d + 1), pl,
                                              16 * kh:16 * (kh + 1), :]
                                    dv = cc2_in[kh][4 * n2h + d, :, pl, :] \
                                        .rearrange("k (m n) -> m k n", m=32, n=B)
                                    eng = nc.sync if (pl + d + kh) % 2 == 0 else nc.scalar
                                    eng.dma_start(dv, sv)
                        nc.gpsimd.collective_compute(
                            "AllToAll", mybir.AluOpType.bypass, replica_groups=rg,
                            ins=[cc2_in[kh][:, :, :, :].rearrange("w k p r -> w (k p r)")],
                            outs=[cc2_out[kh][:, :, :, :].rearrange("w k p r -> w (k p r)")])

            # ============ Phase A': conj(T1), inverse stage A (Im only) ============
            with tc.tile_pool(name="f_in", bufs=2) as f_in, \
                 tc.tile_pool(name="f_tmp", bufs=6) as f_tmp, \
                 tc.tile_pool(name="f_out", bufs=2) as f_outp, \
                 tc.tile_pool(name="f_ps", bufs=2, space="PSUM") as f_ps:
                for c2 in range(NCHUNK // 2):
                    pp2 = []
                    for h in range(2):
                        # partition order p = kh*64 + w_rel*16 + j (n1l = w_rel*32
                        # + 16*kh + j); stage-A tables / t1 / aw rows are
                        # host-permuted to match, so slices stay contiguous
                        t = f_in.tile([128, 2, 2 * CH], BF16, name=f"pin{h}", tag=f"pin{h}")
                        for kh in range(2):
                            eng = nc.sync if (h + kh) % 2 == 0 else nc.scalar
                            eng.dma_start(
                                t[64 * kh:64 * (kh + 1), :, :],
                                cc2_out[kh][4 * h:4 * (h + 1), :, :,
                                            2 * CH * c2:2 * CH * (c2 + 1)])
                        pp2.append(t)
                    o2 = f_outp.tile([128, 2 * CH], F32, tag="fout")
                    for cc in range(2):
                        c = 2 * c2 + cc
                        ps_o = f_ps.tile([128, CH], F32, tag="fps")
                        for h in range(2):
                            pr = pp2[h][:, 0, cc * CH:(cc + 1) * CH]
                            pi = pp2[h][:, 1, cc * CH:(cc + 1) * CH]
                            t1r_t = t1_t[:, h, c, 0, :]
                            t1i_t = t1_t[:, h, c, 1, :]

                            # Yf = P * conj(T1), all-bf16 on DVE (2-byte fast mode)
                            yfr = f_tmp.tile([128, CH], BF16, tag="yf")
                            yfi = f_tmp.tile([128, CH], BF16, tag="yf")
                            p1 = f_tmp.tile([128, CH], BF16, tag="ftmp")
                            p2 = f_tmp.tile([128, CH], BF16, tag="ftmp")
                            p3 = f_tmp.tile([128, CH], BF16, tag="ftmp")
                            p4 = f_tmp.tile([128, CH], BF16, tag="ftmp")
                            nc.vector.tensor_mul(p1[:], pr, t1r_t)
                            nc.vector.tensor_mul(p2[:], pi, t1i_t)
                            nc.vector.tensor_mul(p3[:], pi, t1r_t)
                            nc.vector.tensor_mul(p4[:], pr, t1i_t)
                            nc.vector.tensor_add(yfr[:], p1[:], p2[:])
                            nc.vector.tensor_sub(yfi[:], p3[:], p4[:])

                            st = h == 0
                            sp = h == 1
                            for q in range(2):
                                qs = slice(q * 512, (q + 1) * 512)
                                nc.tensor.matmul(ps_o[:, qs], aw1b[h], yfi[:, qs],
                                                 start=st, stop=False, skip_group_check=True)
                                nc.tensor.matmul(ps_o[:, qs], aw2b[h], yfr[:, qs],
                                                 start=False, stop=sp, skip_group_check=True)
                        nc.scalar.copy(o2[:, cc * CH:(cc + 1) * CH], ps_o[:])
                    eng = nc.sync if c2 % 2 == 0 else nc.scalar
                    eng.dma_start(y_out[:, 2 * CH * c2:2 * CH * (c2 + 1)], o2[:])

    nc.compile()
    return nc


_NC = None
_TABLES = None


def _tables():
    global _TABLES
    if _TABLES is None:
        import ml_dtypes
        k = np.arange(B)
        D = np.exp(-2j * np.pi * np.outer(k, k) / B)
        T2 = np.exp(-2j * np.pi * np.outer(k, k) / R)
        s = 1.0 / (2.0 * M)
        dr = D.real.astype(np.float32)
        di = D.imag.astype(np.float32)
        ndi = -di
        t2r = T2.real.astype(np.float32)
        t2i = T2.imag.astype(np.float32)
        aw1 = (s * dr[:, :128]).astype(np.float32)
        aw2 = (-s * di[:, :128]).astype(np.float32)

        ckr = np.empty((128, 4096), np.float32)
        for p in range(2):
            rs = slice(128 * p, 128 * (p + 1))
            ckr[:, 256 * p:256 * (p + 1)] = dr[rs]
            ckr[:, 512 + 256 * p:512 + 256 * (p + 1)] = di[rs]
            ckr[:, 1024 + 256 * p:1024 + 256 * (p + 1)] = ndi[rs]
            ckr[:, 1536 + 512 * p:1536 + 512 * (p + 1)] = np.concatenate(
                [dr[rs], ndi[rs]], axis=1)
            ckr[:, 2560 + 512 * p:2560 + 512 * (p + 1)] = np.concatenate(
                [di[rs], dr[rs]], axis=1)
            ckr[:, 3584 + 128 * p:3584 + 128 * (p + 1)] = aw1[rs]
            ckr[:, 3840 + 128 * p:3840 + 128 * (p + 1)] = aw2[rs]

        ckf = np.empty((128, 2048), np.float32)
        for p in range(2):
            rs = slice(128 * p, 128 * (p + 1))
            ckf[:, 512 * p:512 * (p + 1)] = np.concatenate([t2r[rs], t2i[rs]], axis=1)
            ckf[:, 1024 + 512 * p:1024 + 512 * (p + 1)] = np.concatenate(
                [t2i[rs], t2r[rs]], axis=1)

        # phase A / A' partition order within each n1-half: p = kh*64 + w*16 + j
        # <-> n1l = w*32 + 16*kh + j  (w = r-shard row block, kh = k1l half)
        pq = np.arange(128)
        n1l = (pq % 64 // 16) * 32 + (pq // 64) * 16 + pq % 16
        ckb = np.empty((128, 3328), np.float32)
        for hh in range(2):
            hs = slice(128 * hh, 128 * (hh + 1))
            ckb[:, 128 * hh:128 * (hh + 1)] = dr[:128, 128 * hh + n1l]
            ckb[:, 256 + 128 * hh:256 + 128 * (hh + 1)] = di[:128, 128 * hh + n1l]
            ckb[:, 512 + 128 * hh:512 + 128 * (hh + 1)] = ndi[:128, 128 * hh + n1l]
        for p in range(2):
            rs = slice(128 * p, 128 * (p + 1))
            ckb[:, 768 + 512 * p:768 + 512 * (p + 1)] = np.concatenate(
                [dr[rs], di[rs]], axis=1)
            ckb[:, 1792 + 512 * p:1792 + 512 * (p + 1)] = np.concatenate(
                [ndi[rs], dr[rs]], axis=1)
            ckb[:, 2816 + 128 * p:2816 + 128 * (p + 1)] = aw1[128 * p + n1l]
            ckb[:, 3072 + 128 * p:3072 + 128 * (p + 1)] = aw2[128 * p + n1l]
        ckb = np.ascontiguousarray(ckb.astype(ml_dtypes.bfloat16))

        t1s = []
        for w in range(W):
            r = np.arange(w * RL, (w + 1) * RL)
            T1 = np.exp(-2j * np.pi * np.outer(k, r) / M)
            t1r = T1.real.astype(np.float32).reshape(2, 128, NCHUNK, CH)
            t1i = T1.imag.astype(np.float32).reshape(2, 128, NCHUNK, CH)
            t1f = np.empty((128, 2, NCHUNK, 2, CH), np.float32)
            t1f[:, :, :, 0, :] = t1r[:, n1l].transpose(1, 0, 2, 3)
            t1f[:, :, :, 1, :] = t1i[:, n1l].transpose(1, 0, 2, 3)
            t1s.append(np.ascontiguousarray(t1f.astype(ml_dtypes.bfloat16)))
        _TABLES = dict(
            ckr=np.ascontiguousarray(ckr),
            ckf=np.ascontiguousarray(ckf),
            ckb=ckb, t1s=t1s,
        )
    return _TABLES


def make_in_maps(a, x):
    tb = _tables()
    a3 = a.reshape(128, W, RL)
    x3 = x.reshape(128, W, RL)
    import ml_dtypes
    in_maps = []
    for c in range(W):
        in_maps.append(dict(
            a_c=np.ascontiguousarray(a3[:, c, :].astype(ml_dtypes.bfloat16)),
            x_c=np.ascontiguousarray(x3[:, c, :].astype(ml_dtypes.bfloat16)),
            t1c=tb["t1s"][c],
            ckr=tb["ckr"], ckf=tb["ckf"], ckb=tb["ckb"],
        ))
    return in_maps


def kernel(a, x, _want_trace=False, **_unused):
    global _NC
    a = np.asarray(a, dtype=np.float32)
    x = np.asarray(x, dtype=np.float32)
    if _NC is None:
        _NC = build_nc()
    in_maps = make_in_maps(a, x)
    res = run_bass_kernel_spmd(_NC, in_maps, core_ids=list(range(W)),
                               trace=_want_trace)
    full = np.empty((128, R), dtype=np.float32)
    for c in range(W):
        full[:, c * RL:(c + 1) * RL] = res.results[c]["y_c"]
    out = full.reshape(-1)
    if _want_trace:
        return out, res
    return out
